# revision 19
# baseline (speedup 1.0000x reference)
import sys
sys.path.insert(0, "/opt/trn_rl_repo")

import numpy as np
import ml_dtypes
from contextlib import ExitStack

import concourse.bass as bass
import concourse.mybir as mybir
import concourse.tile as tile
from concourse import bacc
from concourse.bass_utils import run_bass_kernel_spmd

# ---- problem constants (hardcoded per spec) ----
H, W = 200, 100
NQ, NV, EMB, HEADS, NPT, DH = 2000, 20000, 256, 8, 4, 32
N_CORES = 8
YP = 203                    # padded rows y' = y+1, y' in [0, 202]
NCELL = YP * W              # 20300
NCELL_PAD = 20352           # 159 * 128
NTILE_C = NCELL_PAD // 128  # 159
VEXT = 20452                # 100 zero cols + 20000 real + 352 zero
QT = 2048                   # padded queries (16 tiles of 128)
NQT = 16
F32 = mybir.dt.float32
BF16 = mybir.dt.bfloat16
I16 = mybir.dt.int16

_CACHE = {}


def build_kernel(debug=False):
    nc = bacc.Bacc("TRN2", target_bir_lowering=False, debug=False,
                   num_devices=N_CORES)
    A = mybir.AluOpType
    ACT = mybir.ActivationFunctionType
    dt = nc.dram_tensor
    q_in = dt("q", [NQ, EMB], F32, kind="ExternalInput")
    v_in = dt("v", [NV, EMB], F32, kind="ExternalInput")
    ref_in = dt("ref", [NQ, 2], F32, kind="ExternalInput")
    wv_in = dt("wv", [2, 128, EMB], BF16, kind="ExternalInput")
    woa_in = dt("woa", [2, 128, 96], BF16, kind="ExternalInput")
    boa_in = dt("boa", [96], F32, kind="ExternalInput")
    wout_in = dt("wout", [2, 128, 256], BF16, kind="ExternalInput")
    bout_in = dt("bout", [256], F32, kind="ExternalInput")
    idf_in = dt("idf", [128, 128], F32, kind="ExternalInput")
    idb_in = dt("idb", [128, 128], BF16, kind="ExternalInput")
    outT = dt("outT", [2, 128, NQ], F32, kind="ExternalOutput")

    with tile.TileContext(nc) as tc, ExitStack() as octx:
        const = octx.enter_context(tc.tile_pool(name="const", bufs=1))
        dram = octx.enter_context(tc.tile_pool(name="dram", bufs=1,
                                               space="DRAM"))
        # 4 head-pair planes: plane hp holds, per entry e (cell), the
        # 128-elem record [h2(2), y2(2), d(32)] for heads (2hp, 2hp+1).
        vp2t = dram.tile([4 * NCELL_PAD * 128], BF16, tag="vp2")
        idxd = dram.tile([32 * QT], I16, tag="idxd")

        idf = const.tile([128, 128], F32, tag="idf")
        nc.sync.dma_start(out=idf, in_=idf_in[:, :])
        idb = const.tile([128, 128], BF16, tag="idb")
        nc.sync.dma_start(out=idb, in_=idb_in[:, :])
        wv = [const.tile([128, EMB], BF16, tag=f"wv{ch}", name=f"wv{ch}") for ch in range(2)]
        woa = [const.tile([128, 96], BF16, tag=f"woa{ch}", name=f"woa{ch}") for ch in range(2)]
        wout = [const.tile([128, 256], BF16, tag=f"wo{ch}", name=f"wo{ch}") for ch in range(2)]
        for ch in range(2):
            nc.sync.dma_start(out=wv[ch], in_=wv_in[ch])
            nc.sync.dma_start(out=woa[ch], in_=woa_in[ch])
            nc.sync.dma_start(out=wout[ch], in_=wout_in[ch])
        bias_oa = const.tile([128, 96], F32, tag="boa")
        nc.sync.dma_start(out=bias_oa, in_=bass.AP(
            tensor=boa_in, offset=0, ap=[[0, 128], [1, 96]]))
        bout_sb = const.tile([128, 2], F32, tag="bout")
        nc.sync.dma_start(out=bout_sb, in_=bass.AP(
            tensor=bout_in, offset=0, ap=[[1, 128], [128, 2]]))

        persist = octx.enter_context(tc.tile_pool(name="persist", bufs=1))
        qTf = [persist.tile([128, QT], F32, tag=f"qTf{c}", name=f"qTf{c}") for c in range(2)]
        qTb = [persist.tile([128, QT], BF16, tag=f"qTb{c}", name=f"qTb{c}") for c in range(2)]
        oa_sb = persist.tile([128, NQT, 96], F32, tag="oa")
        wcb = persist.tile([128, NQT, 32, 4], BF16, tag="wcb")
        acc = persist.tile([128, NQT, HEADS, DH], F32, tag="accq")
        ctt = [persist.tile([128, QT], BF16, tag=f"ct{c}", name=f"ct{c}") for c in range(2)]

        # ---------- stage 0: transpose query -> qT ----------
        with tc.tile_pool(name="s0", bufs=3) as s0, \
             tc.tile_pool(name="s0p", bufs=2, space="PSUM") as s0p:
            for g in range(4):
                qt4 = s0.tile([128, 4, 256], F32, tag="q_ld")
                if g == 3:
                    nc.vector.memset(qt4[:, 3, :], 0.0)
                    nc.sync.dma_start(
                        out=qt4[:, 0:3, :],
                        in_=q_in[g * 512:g * 512 + 384, :]
                        .rearrange("(a p) c -> p a c", p=128))
                    nc.sync.dma_start(out=qt4[:NQ - 1920, 3, :],
                                      in_=q_in[1920:NQ, :])
                else:
                    nc.sync.dma_start(
                        out=qt4,
                        in_=q_in[g * 512:(g + 1) * 512, :]
                        .rearrange("(a p) c -> p a c", p=128))
                for ch in range(2):
                    ps = s0p.tile([128, 512], F32, tag="qt_ps")
                    for k in range(4):
                        nc.tensor.transpose(ps[:, k * 128:(k + 1) * 128],
                                            qt4[:, k, ch * 128:(ch + 1) * 128],
                                            idf)
                    nc.scalar.activation(qTf[ch][:, g * 512:(g + 1) * 512],
                                         ps, ACT.Copy)
            for ch in range(2):
                nc.vector.tensor_copy(qTb[ch], qTf[ch])
            for ch in range(2):
                nc.vector.tensor_scalar(out=qTf[ch], in0=qTf[ch],
                                        scalar1=bout_sb[:, ch:ch + 1],
                                        scalar2=None, op0=A.add)

        # ---------- stage 1+2: value transpose, projection, vp2 planes ----
        with tc.tile_pool(name="vtp", bufs=1) as vtp:
            vt = [vtp.tile([128, VEXT], BF16, tag=f"vt{c}", name=f"vt{c}") for c in range(2)]
            for ch in range(2):
                nc.vector.memset(vt[ch][:, 0:100], 0.0)
                nc.vector.memset(vt[ch][:, 20100:VEXT], 0.0)
            with tc.tile_pool(name="s1", bufs=3) as s1, \
                 tc.tile_pool(name="s1p", bufs=4, space="PSUM") as s1p:
                for g in range(40):
                    ntl = 4 if g < 39 else 1
                    vb4 = s1.tile([128, 4, 256], BF16, tag="v_ld")
                    if g < 39:
                        nc.gpsimd.dma_start(
                            out=vb4,
                            in_=v_in[g * 512:(g + 1) * 512, :]
                            .rearrange("(a p) c -> p a c", p=128))
                    else:
                        nc.vector.memset(vb4[:, 0, :], 0.0)
                        nc.gpsimd.dma_start(out=vb4[:32, 0, :],
                                            in_=v_in[19968:NV, :])
                    for ch in range(2):
                        ps = s1p.tile([128, 512], BF16, tag="vt_ps")
                        for k in range(ntl):
                            nc.tensor.transpose(
                                ps[:, k * 128:(k + 1) * 128],
                                vb4[:, k, ch * 128:(ch + 1) * 128], idb)
                        cols = ntl * 128
                        dst = vt[ch][:, 100 + g * 512:100 + g * 512 + cols]
                        if (g * 2 + ch) % 2 == 0:
                            nc.vector.tensor_copy(dst, ps[:, :cols])
                        else:
                            nc.scalar.activation(dst, ps[:, :cols], ACT.Copy)

            # project, convert to head-pair-grouped bf16, write planes
            with tc.tile_pool(name="s2", bufs=3) as s2, \
                 tc.tile_pool(name="s2p", bufs=3, space="PSUM") as s2p:
                NG = 20              # 20 groups of 8 tiles = 160 (159 + pad)
                for gg in range(NG):
                    ntl = 8 if gg < NG - 1 else 7
                    mx8 = s2.tile([128, 8, 4, 128], BF16, tag="mx")
                    for a in range(ntl):
                        ci = gg * 8 + a
                        ps = s2p.tile([128, 512], F32, tag="pj_ps")
                        c0 = ci * 128
                        for ch in range(2):
                            nc.tensor.matmul(ps[:, 0:256],
                                             vt[ch][:, c0:c0 + 128], wv[ch],
                                             start=(ch == 0), stop=(ch == 1))
                        for ch in range(2):
                            nc.tensor.matmul(ps[:, 256:512],
                                             vt[ch][:, c0 + 100:c0 + 228],
                                             wv[ch],
                                             start=(ch == 0), stop=(ch == 1))
                        # ps free layout: (s2, h8, d32) -> dst (hp4, h2, s2, d32)
                        src = ps.rearrange("p (s hp h d) -> p hp h s d",
                                           s=2, hp=4, h=2)
                        dst = mx8[:, a, :, :].rearrange(
                            "p hp (h s d) -> p hp h s d", h=2, s=2)
                        if ci % 2 == 0:
                            nc.vector.tensor_copy(dst, src)
                        else:
                            nc.scalar.activation(dst, src, ACT.Copy)
                    for hp in range(4):
                        base = hp * NCELL_PAD * 128 + gg * 8 * 128 * 128
                        nc.sync.dma_start(
                            out=vp2t[base:base + ntl * 128 * 128]
                            .rearrange("(a p f) -> p a f", p=128, a=ntl),
                            in_=mx8[:, 0:ntl, hp, :])

        # ---------- stage 3: off/attn projections ----------
        with tc.tile_pool(name="s3p", bufs=2, space="PSUM") as s3p:
            for t in range(NQT):
                ps = s3p.tile([128, 96], F32, tag="oa_ps")
                for ch in range(2):
                    nc.tensor.matmul(ps, qTb[ch][:, t * 128:(t + 1) * 128],
                                     woa[ch], start=(ch == 0), stop=(ch == 1))
                nc.vector.tensor_add(oa_sb[:, t, :], ps, bias_oa)

        # ---------- stage 4: coords, weights, indices ----------
        with tc.tile_pool(name="s4", bufs=1) as s4, \
             tc.tile_pool(name="s4p", bufs=2, space="PSUM") as s4p:
            shp = [128, NQT, 32]
            ref_sb = s4.tile([128, NQT, 2], F32, tag="ref")
            nc.vector.memset(ref_sb, 0.0)
            nc.sync.dma_start(
                out=ref_sb[:, 0:15, :],
                in_=ref_in[0:1920, :].rearrange("(t p) c -> p t c", p=128))
            nc.sync.dma_start(out=ref_sb[:NQ - 1920, 15, :],
                              in_=ref_in[1920:NQ, :])
            oav = oa_sb.rearrange("p t (c two) -> p t c two", two=2)
            ox = oav[:, :, 0:32, 0]
            oy = oav[:, :, 0:32, 1]
            awl = oa_sb[:, :, 64:96]

            awe = s4.tile(shp, F32, tag="awe")
            nc.scalar.activation(awe, awl, ACT.Exp)
            s1t = s4.tile([128, NQT, 16], F32, tag="s1t")
            av = awe.rearrange("p t (c two) -> p t c two", two=2)
            nc.vector.tensor_add(s1t, av[:, :, :, 0], av[:, :, :, 1])
            s2t = s4.tile([128, NQT, 8], F32, tag="s2t")
            sv = s1t.rearrange("p t (c two) -> p t c two", two=2)
            nc.vector.tensor_add(s2t, sv[:, :, :, 0], sv[:, :, :, 1])
            rec = s4.tile([128, NQT, 8], F32, tag="rec")
            nc.vector.reciprocal(rec, s2t)
            awn = s4.tile(shp, F32, tag="awn")
            nc.vector.tensor_mul(
                awn.rearrange("p t (c f) -> p t c f", f=4),
                awe.rearrange("p t (c f) -> p t c f", f=4),
                rec[:, :, :, None].broadcast_to([128, NQT, 8, 4]))

            refw = s4.tile([128, NQT, 2], F32, tag="refw")
            nc.vector.tensor_scalar(out=refw[:, :, 0:1],
                                    in0=ref_sb[:, :, 0:1],
                                    scalar1=float(W), scalar2=0.5,
                                    op0=A.mult, op1=A.add)
            nc.vector.tensor_scalar(out=refw[:, :, 1:2],
                                    in0=ref_sb[:, :, 1:2],
                                    scalar1=float(H), scalar2=0.5,
                                    op0=A.mult, op1=A.add)
            px = s4.tile(shp, F32, tag="px")
            nc.vector.tensor_add(px, ox, refw[:, :, 0:1].broadcast_to(shp))
            py = s4.tile(shp, F32, tag="py")
            nc.vector.tensor_add(py, oy, refw[:, :, 1:2].broadcast_to(shp))
            nc.vector.tensor_scalar(out=px, in0=px, scalar1=0.0,
                                    scalar2=float(W + 1),
                                    op0=A.max, op1=A.min)
            nc.vector.tensor_scalar(out=py, in0=py, scalar1=0.0,
                                    scalar2=float(YP - 2),
                                    op0=A.max, op1=A.min)
            M23 = float(1 << 23)
            x0 = s4.tile(shp, F32, tag="x0")
            nc.vector.tensor_scalar(out=x0, in0=px, scalar1=M23 - 0.5,
                                    scalar2=M23, op0=A.add, op1=A.subtract)
            y0 = s4.tile(shp, F32, tag="y0")
            nc.vector.tensor_scalar(out=y0, in0=py, scalar1=M23 - 0.5,
                                    scalar2=M23, op0=A.add, op1=A.subtract)
            fx = s4.tile(shp, F32, tag="fx")
            nc.vector.tensor_sub(fx, px, x0)
            fy = s4.tile(shp, F32, tag="fy")
            nc.vector.tensor_sub(fy, py, y0)

            idxf = s4.tile([128, NQT, 32], F32, tag="idxf")
            cellf = s4.tile(shp, F32, tag="cellf")
            nc.vector.scalar_tensor_tensor(out=cellf, in0=y0,
                                           scalar=float(W), in1=x0,
                                           op0=A.mult, op1=A.add)
            nc.vector.tensor_scalar(out=idxf, in0=cellf,
                                    scalar1=1.0, scalar2=0.0,
                                    op0=A.subtract, op1=A.max)
            nc.vector.tensor_scalar(out=idxf, in0=idxf,
                                    scalar1=float(NCELL - 1), scalar2=None,
                                    op0=A.min)

            ga1 = s4.tile(shp, F32, tag="ga1")
            nc.vector.tensor_scalar(out=ga1, in0=x0, scalar1=0.5,
                                    scalar2=None, op0=A.is_ge)
            ga2 = s4.tile(shp, F32, tag="ga2")
            nc.vector.tensor_scalar(out=ga2, in0=x0, scalar1=float(W) + 0.5,
                                    scalar2=None, op0=A.is_le)
            gb = s4.tile(shp, F32, tag="gb")
            nc.vector.tensor_scalar(out=gb, in0=x0, scalar1=float(W) - 0.5,
                                    scalar2=None, op0=A.is_le)
            fx1 = s4.tile(shp, F32, tag="fx1")
            nc.vector.tensor_scalar(out=fx1, in0=fx, scalar1=-1.0,
                                    scalar2=1.0, op0=A.mult, op1=A.add)
            fy1 = s4.tile(shp, F32, tag="fy1")
            nc.vector.tensor_scalar(out=fy1, in0=fy, scalar1=-1.0,
                                    scalar2=1.0, op0=A.mult, op1=A.add)
            aa = s4.tile(shp, F32, tag="aa")
            nc.vector.tensor_mul(aa, fx1, ga1)
            nc.vector.tensor_mul(aa, aa, ga2)
            nc.vector.tensor_mul(aa, aa, awn)
            bb = s4.tile(shp, F32, tag="bb")
            nc.vector.tensor_mul(bb, fx, gb)
            nc.vector.tensor_mul(bb, bb, awn)

            # corner weights, query-major, bf16: j = x*2 + y
            wcv = wcb.rearrange("p t c j -> p t c j")
            nc.vector.tensor_mul(wcv[:, :, :, 0], aa, fy1)
            nc.vector.tensor_mul(wcv[:, :, :, 1], aa, fy)
            nc.vector.tensor_mul(wcv[:, :, :, 2], bb, fy1)
            nc.vector.tensor_mul(wcv[:, :, :, 3], bb, fy)

            idxT = s4.tile([32, QT], F32, tag="idxT")
            for t in range(NQT):
                ps2 = s4p.tile([128, 128], F32, tag="tr2_ps")
                nc.tensor.transpose(ps2[:32, :], idxf[:, t, :], idf)
                dstv = idxT.rearrange("p (v tt u) -> p v tt u",
                                      v=16, tt=16)[:, :, t, :]
                srcv = ps2[:32, :].rearrange("p (u v) -> p v u", u=8, v=16)
                nc.vector.tensor_copy(dstv, srcv)
            idx16 = s4.tile([32, QT], I16, tag="idx16")
            nc.vector.tensor_copy(idx16, idxT)
            nc.sync.dma_start(
                out=idxd[:].rearrange("(p f) -> p f", p=32), in_=idx16)

        # ---------- stage 5: fused-x gathers + query-major combine --------
        vp2full = vp2t[:]
        with tc.tile_pool(name="s5i", bufs=1) as s5i, \
             tc.tile_pool(name="s5", bufs=3) as s5, \
             tc.tile_pool(name="s5b", bufs=2) as s5b, \
             tc.tile_pool(name="s55p", bufs=2, space="PSUM") as s55p:
            ibs = []
            for call in range(16):
                ib = s5i.tile([128, 256], I16, tag=f"ib{call}",
                              name=f"ib{call}")
                for pr in range(2):
                    row = call * 2 + pr
                    nc.sync.dma_start(
                        out=ib[:, pr * 128:(pr + 1) * 128],
                        in_=bass.AP(
                            tensor=idxd[:].tensor,
                            offset=idxd[:].offset + row * QT,
                            ap=[[0, 8], [128, 16], [1, 128]]))
                ibs.append(ib)
            for h in range(HEADS):
                hp = h // 2
                h2 = h % 2
                for g2 in range(2):
                    call = h * 2 + g2
                    g8 = s5.tile([128, 32, 256], BF16, tag="g8")
                    src_ap = bass.AP(tensor=vp2full.tensor,
                                     offset=vp2full.offset
                                     + hp * NCELL_PAD * 128,
                                     ap=[[128, NCELL_PAD - 2], [1, 256]])
                    if call == 15:
                        for pr2 in range(2):
                            nc.gpsimd.dma_gather(
                                g8[:, pr2 * 16:(pr2 + 1) * 16, :], src_ap,
                                ibs[call][:, pr2 * 128:(pr2 + 1) * 128],
                                QT, QT, 256, elem_step=128,
                                transpose=False, single_packet=False)
                    else:
                        nc.gpsimd.dma_gather(
                            g8, src_ap,
                            ibs[call], 2 * QT, 2 * QT, 256, elem_step=128,
                            transpose=False, single_packet=False)
                    # g8[qp, (pr, qt), (x2, h2, y2, d)]
                    gv = g8.rearrange("q c (x h y d) -> q c x h y d",
                                      x=2, h=2, y=2)
                    for pr in range(2):
                        p = g2 * 2 + pr
                        gq = gv[:, pr * NQT:(pr + 1) * NQT, :, h2, :, :]
                        xs = s5b.tile([128, NQT, 2, DH], F32, tag="xs")
                        for y in range(2):
                            tmp = s5b.tile([128, NQT, 2, DH], F32, tag="tmp")
                            wsel = wcb[:, :, h * 4 + p, y::2]
                            nc.vector.tensor_mul(
                                tmp, gq[:, :, :, y, :],
                                wsel[:, :, :, None]
                                .broadcast_to([128, NQT, 2, DH]))
                            nc.vector.tensor_add(xs[:, :, y, :],
                                                 tmp[:, :, 0, :],
                                                 tmp[:, :, 1, :])
                        if p == 0:
                            nc.vector.tensor_add(acc[:, :, h, :],
                                                 xs[:, :, 0, :],
                                                 xs[:, :, 1, :])
                        else:
                            s_y = s5b.tile([128, NQT, DH], F32, tag="s_y")
                            nc.vector.tensor_add(s_y, xs[:, :, 0, :],
                                                 xs[:, :, 1, :])
                            nc.vector.tensor_add(acc[:, :, h, :],
                                                 acc[:, :, h, :], s_y)
                # head h complete: transpose its 32 output dims now
                half = h // 4
                hb = (h % 4) * 32
                for t in range(NQT):
                    ps = s55p.tile([128, 128], F32, tag="ct_ps")
                    nc.tensor.transpose(ps[:32, :], acc[:, t, h, :], idf)
                    nc.scalar.activation(
                        ctt[half][hb:hb + 32, t * 128:(t + 1) * 128],
                        ps[:32, :], ACT.Copy)

        # ---------- stage 6: output projection + identity ----------
        with tc.tile_pool(name="s6", bufs=2) as s6, \
             tc.tile_pool(name="s6p", bufs=2, space="PSUM") as s6p:
            for oh in range(2):
                ps = s6p.tile([128, QT], F32, tag="out_ps")
                for qc in range(4):
                    for ch in range(2):
                        nc.tensor.matmul(
                            ps[:, qc * 512:(qc + 1) * 512],
                            wout[ch][:, oh * 128:(oh + 1) * 128],
                            ctt[ch][:, qc * 512:(qc + 1) * 512],
                            start=(ch == 0), stop=(ch == 1))
                ot = s6.tile([128, NQ], F32, tag="ot")
                nc.vector.tensor_add(ot, ps[:, 0:NQ], qTf[oh][:, 0:NQ])
                nc.sync.dma_start(out=outT[oh], in_=ot)

    nc.finalize()
    return nc


def _prep_shared(inputs):
    bf = ml_dtypes.bfloat16
    W_val = np.asarray(inputs["W_val"], np.float32)
    W_off = np.asarray(inputs["W_off"], np.float32)
    W_attn = np.asarray(inputs["W_attn"], np.float32)
    W_out = np.asarray(inputs["W_out"], np.float32)
    b_off = np.asarray(inputs["b_off"], np.float32)
    b_attn = np.asarray(inputs["b_attn"], np.float32)
    b_val = np.asarray(inputs["b_val"], np.float32)
    b_out = np.asarray(inputs["b_out"], np.float32)
    assert np.allclose(b_val, 0.0), "kernel assumes b_val == 0"
    woa = np.concatenate([W_off, W_attn], axis=1)
    boa = np.concatenate([b_off, b_attn], axis=0)
    idf = np.eye(128, dtype=np.float32)
    return dict(
        wv=np.ascontiguousarray(W_val.reshape(2, 128, 256)).astype(bf),
        woa=np.ascontiguousarray(woa.reshape(2, 128, 96)).astype(bf),
        boa=boa,
        wout=np.ascontiguousarray(W_out.reshape(2, 128, 256)).astype(bf),
        bout=b_out,
        idf=idf, idb=idf.astype(bf))


def make_in_maps(inputs):
    shared = _prep_shared(inputs)
    q = np.asarray(inputs["query"], np.float32)
    v = np.asarray(inputs["value"], np.float32)
    ref = np.asarray(inputs["reference_points"], np.float32)
    in_maps = []
    for c in range(N_CORES):
        in_maps.append(dict(
            q=np.ascontiguousarray(q[:, c, :]),
            v=np.ascontiguousarray(v[:, c, :]),
            ref=np.ascontiguousarray(ref[c, :, 0, :]),
            **shared))
    return in_maps


def post(results, inputs):
    out = np.empty((NQ, N_CORES, EMB), np.float32)
    for c in range(N_CORES):
        oT = results[c]["outT"]
        out[:, c, :] = oT.reshape(256, NQ).T
    return out


def kernel(**inputs):
    if "nc" not in _CACHE:
        _CACHE["nc"] = build_kernel(debug=False)
    nc = _CACHE["nc"]
    in_maps = make_in_maps(inputs)
    res = run_bass_kernel_spmd(nc, in_maps, core_ids=list(range(N_CORES)))
    return post(res.results, inputs)
